# revision 4
# baseline (speedup 1.0000x reference)
"""Trainium2 Bass kernel for nn_DeepConv1d, v2 (self-contained).

Math (per batch b):
  xr   = linear-interp(deep, 1024 -> 4096)           # commutes with 1x1 conv
  y    = conv_w @ xr                                 # conv_b cancels in (y-mean)
  xs   = GAMA*(y-mean)/(var_unbiased+EPS)
  loss_k[c,l] = sech^2(xs_pad[c,l+k]-xs_pad[c,l+3])  # k=0..6, reflect pad 3
  S    = sum_k loss_k ;  W_k = (loss_k/S)*x_pad[:,l+k]
  out[o,l] = sum_{c,k} fc_w[o, 7c+k] * W_k[c,l]

v2 structure (vs v1):
  - Stats without materialized y^2: sum(y) = 4*sum(ys) and
    sum(y^2) = 4*sum(ys^2) - (11/16)*sum(D^2), from telescoping the
    4x-upsample identity.  (D = first differences of ys.)
  - sech^2(d) = 4*sigmoid'(2d); lv = (sa-1)*sa (negated sigma'), with sa
    stored bf16 so the STT runs in DVE 2x mode.  The 4 and the sign fold
    into host-scaled fc (fc_k *= -4 for k != 3).
  - 1/S fused into ONE ACT op per chunk: G = Reciprocal(-4*msum + 1),
    output bf16.  G = 1/S directly (no 4/S), no S32/G4 DVE ops.
  - Shifted (odd-offset) operand copies (ypad1, lv1s, lv3s) made by
    SBUF->SBUF DMA: zero engine time, keeps every TT op 4B-aligned 2x.
  - GEMM: per 512-chunk, taps interleave batch0/batch1 matmuls; the two
    batches contract rows 0:64 / 64:128 -> distinct row-groups of the PE
    array -> they run concurrently (row tiling).  Warm-up matmuls keep
    the HAM clock-gate open before the GEMM burst.
  - PSUM eviction on ACT (closer to PSUM), output bf16 to halve DMA.
"""
import contextlib

import numpy as np
import ml_dtypes

import concourse.bass as bass
import concourse.bacc as bacc_mod
import concourse.mybir as mybir
import concourse.tile as tile
from concourse.bass_utils import run_bass_kernel_spmd

bf16 = ml_dtypes.bfloat16
AF = mybir.ActivationFunctionType
ALU = mybir.AluOpType

KS = 7
PAD = 3
GAMA = 0.5
EPS = 1e-9
N = 4096
ND = 1024
NP = N + 2 * PAD       # 4102
L3 = N + PAD           # 4099: gap-array length
NCORES = 8
NCH = 4                # l-chunks
CW = N // NCH          # 1024

F32 = mybir.dt.float32
BF = mybir.dt.bfloat16

# engine knobs: 'v' = DVE, 'g' = GPSIMD for each splittable op.
# GPSIMD shares its SBUF port with the DVE: concurrent Pool traffic
# drops 2x-mode DVE ops to ~0.42 elem/ns (measured), a net loss — so
# everything stays on the DVE and GPSIMD idles.
OP_ENG = {
    "DD21": "v", "y4a": "v",
    "dy1": "v", "dy2b": "v", "dy3": "v",
    "m1c": "v", "m2": "v", "m3": "v", "s12": "v",
    "P12": "v", "P21": "v", "P30": "v",
    "GL1": "v", "GL2": "v", "GL3": "v",
    "W0": "v", "W1": "v", "W2": "v", "W3": "v", "W4": "v",
    "W5": "v", "W6": "v", "msum": "v",
}
N_WARM_MM = 12


def kernel_body(tc, xp_d, dpq_d, ysq_d, fck_d, out_d):
    nc = tc.nc

    def eng(key):
        return nc.vector if OP_ENG[key] == "v" else nc.gpsimd

    def tt(key, out, in0, in1, op):
        e = eng(key)
        if op == "add":
            e.tensor_add(out=out, in0=in0, in1=in1)
        elif op == "sub":
            e.tensor_sub(out=out, in0=in0, in1=in1)
        else:
            e.tensor_mul(out=out, in0=in0, in1=in1)

    ctx = contextlib.ExitStack()
    with ctx:
        io = ctx.enter_context(tc.tile_pool(name="io", bufs=1))
        mid = ctx.enter_context(tc.tile_pool(name="mid", bufs=1))
        loss = ctx.enter_context(tc.tile_pool(name="loss", bufs=1))
        ck = ctx.enter_context(tc.tile_pool(name="ck", bufs=2))
        stp = ctx.enter_context(tc.tile_pool(name="stp", bufs=2))
        pp = ctx.enter_context(tc.tile_pool(name="pp", bufs=1, space="PSUM"))
        ppa = ctx.enter_context(tc.tile_pool(name="ppa", bufs=3, space="PSUM"))

        # ---------------- input DMAs (small first) ----------------
        dpq = io.tile([128, ND + 2], BF, tag="dpq")
        nc.sync.dma_start(out=dpq, in_=dpq_d[:, :])
        ysq = io.tile([128, ND + 1], F32, tag="ysq")
        nc.sync.dma_start(out=ysq, in_=ysq_d[:, :])
        fck = io.tile([128, KS, 128], BF, tag="fck")
        nc.sync.dma_start(out=fck, in_=fck_d[:, :, :])
        xp = io.tile([128, NP], BF, tag="xp")          # x reflect-padded
        xs1 = io.tile([128, NP - 1], BF, tag="xs1")    # same, shifted 1 elem
        nc.sync.dma_start(out=xp, in_=xp_d[:, :])
        nc.sync.dma_start(out=xs1, in_=xp_d[:, 1:NP])

        ys = ysq[:, 0:ND]
        Dp = dpq[:, 0:ND + 1]
        f2p = ysq[:, ND:ND + 1]
        warm = mid.tile([128, 1], F32, tag="warm")
        nc.scalar.activation(out=warm, in_=dpq[:, 0:1], func=AF.Sigmoid,
                             scale=1.0)
        ys_ps = pp.tile([128, ND], F32, tag="ysps")  # S32/warm scratch

        def _rep2(ap_, off):
            return bass.AP(tensor=ap_.tensor, offset=ap_.offset + off,
                           ap=[list(ap_.ap[0]), [1, ND], [0, 2]])

        def _bcast(ap_):
            return bass.AP(tensor=ap_.tensor, offset=ap_.offset,
                           ap=[list(ap_.ap[0]), [0, ND], [1, 2]])

        # DD pair products live in the loss pool so the sigmoid-square
        # scratch (sq1) can reuse the buffer once interp is done.
        # Computed on ACT as scaled copies (strided writes are free there,
        # and the DVE is the bottleneck engine).
        DDbig = loss.tile([128, 2, ND + 2, 2], BF, tag="Q1", name="DD")
        DD12 = DDbig[:, 0, 0:ND, :]
        DD21 = DDbig[:, 1, 0:ND, :]
        nc.scalar.activation(out=DD12[:, :, 0], in_=Dp[:, 0:ND],
                             func=AF.Copy, scale=0.375)
        nc.scalar.activation(out=DD12[:, :, 1], in_=Dp[:, 0:ND],
                             func=AF.Copy, scale=0.125)
        nc.scalar.activation(out=DD21[:, :, 0], in_=Dp[:, 1:ND + 1],
                             func=AF.Copy, scale=0.125)
        nc.scalar.activation(out=DD21[:, :, 1], in_=Dp[:, 1:ND + 1],
                             func=AF.Copy, scale=0.375)

        # interp + shifted copy in l-quarters so the dy/sigmoid ladder
        # starts ~3 quarters earlier
        ypad = mid.tile([128, NP], BF, tag="ypad")
        y4 = ypad[:, PAD:PAD + N].rearrange("p (j r) -> p j r", r=4)
        ypad1 = loss.tile([128, NP - 1], BF, tag="Q2", name="ypad1")
        JQ = ND // 4

        def _rep2q(ap_, off, q):
            return bass.AP(tensor=ap_.tensor,
                           offset=ap_.offset + off + q * JQ,
                           ap=[list(ap_.ap[0]), [1, JQ], [0, 2]])

        for q in range(4):
            js = slice(q * JQ, (q + 1) * JQ)
            nc.vector.tensor_sub(out=y4[:, js, 0:2], in0=_rep2q(ys, 0, q),
                                 in1=DD12[:, js, :])
            nc.vector.tensor_add(out=y4[:, js, 2:4], in0=_rep2q(ys, 0, q),
                                 in1=DD21[:, js, :])
            if q == 0:
                for i in range(3):   # left reflect: ypad[2-i] = ypad[4+i]
                    nc.vector.tensor_copy(out=ypad[:, 2 - i:3 - i],
                                          in_=ypad[:, 4 + i:5 + i])
            if q == 3:
                for i in range(3):
                    nc.vector.tensor_copy(
                        out=ypad[:, N + 3 + i:N + 4 + i],
                        in_=ypad[:, N + 1 - i:N + 2 - i])
            qe = 4 * JQ * (q + 1) + 1 if q < 3 else NP
            nc.sync.dma_start(out=ypad1[:, 4 * JQ * q:qe - 1],
                              in_=ypad[:, 4 * JQ * q + 1:qe])

        # ---------------- gap diffs (bf16, all 4B-aligned) ------------
        # quarter boundaries at 1022 keep each piece inside the already-
        # DMA'd ypad1 range (reads reach a+2 .. b+2 <= next 1024 boundary)
        dy1 = loss.tile([128, L3], BF, tag="T1")
        dy2b = loss.tile([128, L3], BF, tag="T2")
        dy3 = loss.tile([128, L3], BF, tag="T3")
        QB = [(0, 1022), (1022, 2044), (2044, 3066), (3066, L3)]
        for a, b_ in QB:
            tt("dy1", dy1[:, a:b_], ypad1[:, a:b_], ypad[:, a:b_], "sub")
            tt("dy2b", dy2b[:, a:b_], ypad1[:, a + 2:b_ + 2],
               ypad1[:, a:b_], "sub")
            tt("dy3", dy3[:, a:b_], ypad1[:, a + 2:b_ + 2],
               ypad[:, a:b_], "sub")

        # ---------------- sigmoids (ACT) + lv = sa^2 - sa ----------------
        # lv1/lv2b use an ACT Square helper so the DVE side is a 2x-mode
        # subtract; lv3 (critical path into msum) is a single DVE STT.
        # Everything runs in 2048-wide halves so the DVE starts ~4us
        # earlier instead of waiting out the whole ACT block.
        halves = QB   # quarter-granular, matching the dy boundaries
        sa1 = loss.tile([128, L3], BF, tag="S1")
        sa2 = loss.tile([128, L3], BF, tag="S2")
        sa3 = loss.tile([128, L3], BF, tag="S3")
        sq1 = loss.tile([128, L3], BF, tag="Q1", name="sq1")  # over DD
        sq2 = loss.tile([128, L3], BF, tag="Q2", name="sq2")  # over ypad1
        lv1 = loss.tile([128, L3], BF, tag="T1", name="lv1")  # over dy1
        lv2b = loss.tile([128, L3], BF, tag="T2", name="lv2b")
        lv3 = loss.tile([128, L3], BF, tag="T3", name="lv3")
        for a, b_ in halves:
            nc.scalar.activation(out=sa1[:, a:b_], in_=dy1[:, a:b_],
                                 func=AF.Sigmoid, scale=f2p)
            nc.scalar.activation(out=sq1[:, a:b_], in_=sa1[:, a:b_],
                                 func=AF.Square)
        lv1s = loss.tile([128, L3 - 1], BF, tag="S1L")
        for a, b_ in halves:
            nc.vector.tensor_sub(out=lv1[:, a:b_], in0=sq1[:, a:b_],
                                 in1=sa1[:, a:b_])
            nc.sync.dma_start(out=lv1s[:, max(a - 1, 0):b_ - 1],
                              in_=lv1[:, max(a, 1):b_])
        for a, b_ in halves:
            nc.scalar.activation(out=sa2[:, a:b_], in_=dy2b[:, a:b_],
                                 func=AF.Sigmoid, scale=f2p)
            nc.scalar.activation(out=sq2[:, a:b_], in_=sa2[:, a:b_],
                                 func=AF.Square)
        for a, b_ in halves:
            nc.vector.tensor_sub(out=lv2b[:, a:b_], in0=sq2[:, a:b_],
                                 in1=sa2[:, a:b_])
        for a, b_ in halves:
            nc.scalar.activation(out=sa3[:, a:b_], in_=dy3[:, a:b_],
                                 func=AF.Sigmoid, scale=f2p)
        lv3s = loss.tile([128, L3 - 1], BF, tag="S3L")
        for a, b_ in halves:
            nc.vector.scalar_tensor_tensor(
                out=lv3[:, a:b_], in0=sa3[:, a:b_], scalar=1.0,
                in1=sa3[:, a:b_], op0=ALU.subtract, op1=ALU.mult)
            nc.sync.dma_start(out=lv3s[:, max(a - 1, 0):b_ - 1],
                              in_=lv3[:, max(a, 1):b_])

        # PE warm-up: keep HAM open ahead of the GEMM burst (depends on
        # lv3 — never overwritten — so it lands a few us before the first
        # chunk's matmuls without WAR-blocking real work).
        for w in range(N_WARM_MM):
            wdump = pp.tile([128, 512], F32, tag="ysps", name=f"wd{w}")
            nc.tensor.matmul(out=wdump, lhsT=fck[:, 0, :],
                             rhs=lv3[:, 0:512], start=True, stop=True)

        # ------- S partials + P + G, chunk-pipelined ahead of W/GEMM -----
        # G = 1/S: ACT affine (S = -4*msum + 1, fp32, into PSUM) -> DVE
        # fast reciprocal -> ACT cast to bf16.  Identity and Copy live in
        # the sigmoid table set: no table switches.  G tiles park in the
        # dead ypad/Dp buffers.
        m2 = loss.tile([128, N], BF, tag="M2")
        m1c = loss.tile([128, N], BF, tag="M1")
        s12 = loss.tile([128, N], BF, tag="M4", name="s12")
        m3 = loss.tile([128, N], BF, tag="M3")
        msum = loss.tile([128, N], BF, tag="Q1", name="msum")  # over sq1
        P12 = loss.tile([128, N], BF, tag="S1")   # after sa1 consumed
        P21 = loss.tile([128, N], BF, tag="S2")
        P30 = loss.tile([128, N], BF, tag="S3")
        G01 = mid.tile([128, 2, CW], BF, tag="ypad", name="G01")
        G23 = mid.tile([128, 2, CW], BF, tag="Dp", name="G23")
        Gs = [G01[:, 0], G01[:, 1], G23[:, 0], G23[:, 1]]
        for c in range(NCH):
            lo = c * CW
            hi = lo + CW
            tt("m2", m2[:, lo:hi], lv2b[:, lo:hi], lv2b[:, lo + 2:hi + 2],
               "add")
            tt("m1c", m1c[:, lo:hi], lv1[:, lo + 2:hi + 2],
               lv1s[:, lo + 2:hi + 2], "add")
            tt("s12", s12[:, lo:hi], m2[:, lo:hi], m1c[:, lo:hi], "add")
            tt("m3", m3[:, lo:hi], lv3[:, lo:hi], lv3s[:, lo + 2:hi + 2],
               "add")
            tt("msum", msum[:, lo:hi], s12[:, lo:hi], m3[:, lo:hi], "add")
            S32 = pp.tile([128, CW], F32, tag="ysps", name=f"S32_{c}")
            nc.scalar.activation(out=S32, in_=msum[:, lo:hi],
                                 func=AF.Identity, bias=1.0, scale=-4.0)
            if c == 0:   # full-width: P only feeds W0-2, no need to chunk
                tt("P12", P12, lv1[:, 2:N + 2], xp[:, 2:N + 2], "mul")
                tt("P21", P21, lv2b[:, 0:N], xs1[:, 0:N], "mul")
                tt("P30", P30, lv3[:, 0:N], xp[:, 0:N], "mul")
            G32 = ck.tile([128, CW], F32, tag="gtmp", name=f"G32_{c}")
            nc.vector.reciprocal_approx_fast(out=G32, in_=S32)
            nc.scalar.copy(out=Gs[c], in_=G32)

        # ---------------- chunked GL/W -> GEMM -> out ------
        for c in range(NCH):
            lo = c * CW
            G = Gs[c]
            GL1 = ck.tile([128, CW], BF, tag="GL1", name=f"GL1_{c}")
            GL2 = ck.tile([128, CW], BF, tag="GL2", name=f"GL2_{c}")
            GL3 = ck.tile([128, CW], BF, tag="GL3", name=f"GL3_{c}")
            tt("GL1", GL1, lv1s[:, lo + 2:lo + 2 + CW], G, "mul")
            tt("GL2", GL2, lv2b[:, lo + 2:lo + 2 + CW], G, "mul")
            tt("GL3", GL3, lv3s[:, lo + 2:lo + 2 + CW], G, "mul")

            W = [ck.tile([128, CW], BF, tag=f"W{k}", name=f"W{k}_{c}")
                 for k in range(KS)]
            tt("W0", W[0], G, P30[:, lo:lo + CW], "mul")
            tt("W1", W[1], G, P21[:, lo:lo + CW], "mul")
            tt("W2", W[2], G, P12[:, lo:lo + CW], "mul")
            tt("W3", W[3], G, xs1[:, lo + 2:lo + 2 + CW], "mul")
            tt("W4", W[4], GL1, xp[:, lo + 4:lo + 4 + CW], "mul")
            tt("W5", W[5], GL2, xs1[:, lo + 4:lo + 4 + CW], "mul")
            tt("W6", W[6], GL3, xp[:, lo + 6:lo + 6 + CW], "mul")

            acc = [ppa.tile([128, CW], F32, tag="acc",
                            name=f"acc_{c}_{b}") for b in range(2)]
            # k-outer, batches interleaved: the b0/b1 matmuls target
            # disjoint PE row-groups and run concurrently; one tap's
            # weights serve both 512-subs.
            for k in range(KS):
                for sub in range(CW // 512):
                    cs = slice(sub * 512, (sub + 1) * 512)
                    for b in range(2):
                        prow = slice(64 * b, 64 * (b + 1))
                        nc.tensor.matmul(
                            out=acc[b][:, cs],
                            lhsT=fck[prow, k, :],
                            rhs=W[k][prow, cs],
                            start=(k == 0), stop=(k == KS - 1),
                        )
            for b in range(2):
                if c == NCH - 1:
                    # last chunk: 512-wide stages so the final DMA
                    # starts half an eviction earlier
                    for sub in range(2):
                        st = stp.tile([128, 512], BF, tag=f"sst{sub}",
                                      name=f"sstage_{b}_{sub}")
                        nc.scalar.copy(out=st,
                                       in_=acc[b][:, sub * 512:(sub + 1) * 512])
                        nc.sync.dma_start(
                            out=out_d[:, b, lo + sub * 512:lo + (sub + 1) * 512],
                            in_=st)
                else:
                    stage = stp.tile([128, CW], BF, tag="stage",
                                     name=f"stage_{c}_{b}")
                    nc.scalar.copy(out=stage, in_=acc[b])
                    nc.sync.dma_start(out=out_d[:, b, lo:lo + CW], in_=stage)


def build_nc():
    nc = bacc_mod.Bacc(None, target_bir_lowering=False)
    xp_d = nc.dram_tensor("xp", [128, NP], BF, kind="ExternalInput")
    dpq_d = nc.dram_tensor("dpq", [128, ND + 2], BF, kind="ExternalInput")
    ysq_d = nc.dram_tensor("ysq", [128, ND + 1], F32, kind="ExternalInput")
    fck_d = nc.dram_tensor("fck", [128, KS, 128], BF, kind="ExternalInput")
    out_d = nc.dram_tensor("out", [128, 2, N], BF, kind="ExternalOutput")
    with tile.TileContext(nc) as tc:
        kernel_body(tc, xp_d, dpq_d, ysq_d, fck_d, out_d)
    nc.compile()
    return nc


def prep_inputs(deep, x, conv_w, conv_b, fc_w):
    deep = np.asarray(deep, np.float32)
    x = np.asarray(x, np.float32)
    conv_w = np.asarray(conv_w, np.float32)
    fc_w = np.asarray(fc_w, np.float32)

    xpad = np.pad(x, ((0, 0), (0, 0), (PAD, PAD)), mode="reflect")
    xp_all = np.ascontiguousarray(xpad.reshape(NCORES, 128, NP)).astype(bf16)
    # host-side 1x1 conv + exact stats (0.3% of total FLOPs)
    ys_all = np.einsum('cd,bdn->bcn', conv_w, deep)          # (16, 64, 1024)
    Dp_all = np.zeros((16, 64, ND + 1), np.float32)
    Dp_all[:, :, 1:ND] = ys_all[:, :, 1:] - ys_all[:, :, :-1]
    sum_y = 4.0 * ys_all.sum(axis=2)
    sum_y2 = 4.0 * (ys_all ** 2).sum(axis=2) - 0.6875 * (Dp_all ** 2).sum(axis=2)
    var = (sum_y2 - sum_y * sum_y / N) / (N - 1)
    f2p_all = (1.0 / (var + EPS)).astype(np.float32)         # 2*GAMA = 1
    dpq_all = np.concatenate(
        [Dp_all, np.zeros((16, 64, 1), np.float32)],
        axis=2).astype(bf16).reshape(NCORES, 128, ND + 2)
    ysq_all = np.concatenate(
        [ys_all, f2p_all[:, :, None]],
        axis=2).astype(np.float32).reshape(NCORES, 128, ND + 1)

    fc3 = fc_w.reshape(128, 64, KS)
    fck_half = np.transpose(fc3, (1, 2, 0)).copy()
    fck_half *= -4.0              # lv negated + sech^2 = 4*sigmoid'
    fck_half[:, PAD, :] *= -0.25  # center tap: W_3 = G*x directly
    fck = np.ascontiguousarray(
        np.concatenate([fck_half, fck_half], axis=0)).astype(bf16)

    return [{"xp": np.ascontiguousarray(xp_all[ci]),
             "dpq": np.ascontiguousarray(dpq_all[ci]),
             "ysq": np.ascontiguousarray(ysq_all[ci]),
             "fck": fck} for ci in range(NCORES)]


def gather_out(results):
    out_full = np.empty((16, 128, N), np.float32)
    for ci in range(NCORES):
        o = np.asarray(results[ci]["out"], dtype=np.float32)
        out_full[2 * ci] = o[:, 0]
        out_full[2 * ci + 1] = o[:, 1]
    return out_full


_CACHED = {}


def _get_nc():
    if "nc" not in _CACHED:
        _CACHED["nc"] = build_nc()
    return _CACHED["nc"]


def kernel(deep, x, conv_w, conv_b, fc_w):
    in_maps = prep_inputs(deep, x, conv_w, conv_b, fc_w)
    nc = _get_nc()
    res = run_bass_kernel_spmd(nc, in_maps, core_ids=list(range(NCORES)))
    return gather_out(res.results)
